# revision 4
# baseline (speedup 1.0000x reference)
# MoE layer (top-2 routing, degenerate capacity C=2) on 8 TRN2 NeuronCores.
#
# Math (reference collapses over the capacity axis since the dispatch mask is
# broadcast identically into both capacity slots):
#   scores = softmax(x @ Wg + bg)                      [G,S,E]
#   top-2 per token -> dm (0/1 mask), cw = 2 * softmax(top2 scores) scattered
#   D[e,g,:]  = sum_s dm[g,s,e] * x[g,s,:]             (dispatch, per group)
#   h[e,g,:]  = silu(D[e,g,:] @ wi[e].T)
#   eo[e,g,:] = h[e,g,:] @ wo[e].T
#   out[g,s,:] = sum_e cw[g,s,e] * eo[e,g,:]
#
# Sharding: core c owns group g=c for gating/dispatch/combine and expert e=c
# for the FFN; tiny AllToAlls (E==G==n_cores==8) redistribute the [8,M]
# dispatched / expert-output rows between the two phases.
#
# v2 vs the 392us baseline — all aimed at the DMA roofline (per-core bytes):
#   - x loaded ONCE as a bf16 hi/lo pair (16.8MB) instead of fp32 + bf16
#     (25.2MB). Gating logits = x_hi@Wg_hi + x_hi@Wg_lo + x_lo@Wg_hi in bf16
#     matmuls with fp32 PSUM accumulation, preserving ~16 mantissa bits of
#     the fp32 logits so top-2 routing matches the fp32 reference exactly
#     (measured: 0/16384 flipped tokens); dispatch consumes x_hi directly.
#   - Output written bf16 (8.4MB instead of 16.8MB), upcast on host.
#   - Weights host-pre-laid-out so every weight DMA is one fully-contiguous
#     [128, 8192] 2MB transfer (wi: 16 slabs; wo: 16 slabs in m-column-major
#     order so FFN2 -> AllToAll -> combine pipelines per 512-col m-chunk).
#   - Dispatch AllToAll carried in bf16.
# Per-core DMA: 16.8 (x) + 33.5 (wi) + 33.5 (wo) + 8.4 (out) = 92.3 MB.

import os
from contextlib import ExitStack

import numpy as np
import ml_dtypes

import concourse.bass as bass
from concourse import bacc
import concourse.mybir as mybir
import concourse.tile as tile
from concourse.bass import ts, broadcast_tensor_aps
from concourse.masks import make_identity

F32 = mybir.dt.float32
BF16 = mybir.dt.bfloat16
AF = mybir.ActivationFunctionType
ALU = mybir.AluOpType
AX = mybir.AxisListType

P = 128

G_FULL, S_FULL, M_FULL, H_FULL, E_FULL = 8, 2048, 2048, 8192, 8
N_CORES = 8

LAST_RESULT = None  # BassKernelResults of the most recent device run (for test.py)


def build_bass(S=S_FULL, M=M_FULL, H=H_FULL, E=E_FULL, n_cores=N_CORES):
    assert E == n_cores, "AllToAll layout assumes E == n_cores"
    SB, MO, HB = S // P, M // P, H // P
    HSUP = 512                  # wi slab width along H (16 slabs of 2MB)
    NSUP = H // HSUP
    MCH = 512                   # FFN2 / combine m-chunk
    MC = M // MCH
    WOH = 4                     # wo slabs per m-chunk (2MB each)
    OHB = HB // WOH             # h-blocks per wo slab

    nc = bacc.Bacc(num_devices=n_cores)
    rg = [list(range(n_cores))]

    # host-prepped layouts (see prepare_in_maps)
    xh = nc.declare_dram_parameter("xh", [S, M], BF16, False)
    xl = nc.declare_dram_parameter("xl", [S, M], BF16, False)
    wgc = nc.declare_dram_parameter("wgc", [P, MO, 2 * E], BF16, False)
    bgp = nc.declare_dram_parameter("bgp", [1, 2 * E], F32, False)
    wiT = nc.declare_dram_parameter("wiT", [NSUP * P, MO * HSUP], BF16, False)
    woT = nc.declare_dram_parameter("woT", [MC * WOH * P, OHB * MCH], BF16, False)
    out = nc.declare_dram_parameter("out", [P, SB, M], BF16, True)

    with tile.TileContext(nc) as tc, ExitStack() as stack:
        # ---------- persistent pools ----------
        const_pool = stack.enter_context(tc.tile_pool(name="const", bufs=1))
        ident_f = const_pool.tile([P, P], F32, name="ident_f")
        make_identity(nc, ident_f)
        ident_b = const_pool.tile([P, P], BF16, name="ident_b")
        nc.vector.tensor_copy(ident_b[:], ident_f[:])
        ones1 = const_pool.tile([1, P], F32, name="ones1")
        nc.vector.memset(ones1[:], 1.0)
        wg_sb = const_pool.tile([P, MO, 2 * E], BF16, name="wg_sb")
        nc.scalar.dma_start(wg_sb[:], wgc[:])
        bg_sb = const_pool.tile([1, 2 * E], F32, name="bg_sb")
        nc.scalar.dma_start(bg_sb[:], bgp[:])

        keep_pool = stack.enter_context(tc.tile_pool(name="keep", bufs=1))
        cwT_sb = keep_pool.tile([E, SB, P], BF16, name="cwT_sb")
        dt_sb = keep_pool.tile([P, MO, E], BF16, name="dt_sb")
        ht_sb = keep_pool.tile([P, HB, E], BF16, name="ht_sb")

        # weight streaming pools (opened early so prefetch DMAs have no deps)
        wi_pool = stack.enter_context(tc.tile_pool(name="wi", bufs=4))
        wo_pool = stack.enter_context(tc.tile_pool(name="wo", bufs=4))

        dram = stack.enter_context(tc.tile_pool(name="dram", bufs=1, space="DRAM"))
        d_in = dram.tile([E, M], BF16, name="d_in")
        d_out = dram.tile([E, M], BF16, name="d_out")
        eo_in = [dram.tile([E, MCH], BF16, name=f"eo_in{i}") for i in range(MC)]
        eo_out = [dram.tile([E, MCH], BF16, name=f"eo_out{i}") for i in range(MC)]

        # ---------- phase A: gating + dispatch (group-parallel) ----------
        with (
            tc.tile_pool(name="xa", bufs=2) as xa,
            tc.tile_pool(name="xt", bufs=2) as xt,
            tc.tile_pool(name="sp", bufs=2) as sp,
            tc.tile_pool(name="psA", bufs=1, space="PSUM") as psA,
            tc.tile_pool(name="psD", bufs=1, space="PSUM") as psD,
        ):
            # dispatch accumulator [E, M]: chunks are full PSUM banks --
            # interleaved accumulation chains are only safe at bank granularity
            d_ps = psD.tile([E, M], F32, name="d_ps")
            # Warm-up dummies: absorb identity/wg ticks into PE's vector clock.
            ptd = psA.tile([P, 4 * P], BF16, tag="pst", bufs=3, name="ptd")
            nc.tensor.transpose(ptd[:, :P], ident_b[:], ident_b[:])
            dmy0 = psA.tile([E, E], F32, tag="gA", bufs=1, name="dmy0")
            nc.tensor.matmul(dmy0[:], lhsT=wg_sb[:, 0, :E], rhs=wg_sb[:, 0, :E],
                             start=True, stop=True)
            KB = 4
            for grp in range(SB // KB):
                sbs = [grp * KB + j for j in range(KB)]
                psG = psA.tile([P, KB, E], F32, tag="gA", bufs=1,
                               name=f"gA{grp}")
                xh_ts, dm_bs = [], None
                for j, sb in enumerate(sbs):
                    xh_t = xa.tile([P, M], BF16, tag="xh", bufs=6, name=f"xh{sb}")
                    nc.sync.dma_start(xh_t[:], xh[ts(sb, P), :])
                    xl_t = xa.tile([P, M], BF16, tag="xl", bufs=3, name=f"xl{sb}")
                    nc.sync.dma_start(xl_t[:], xl[ts(sb, P), :])
                    xh_ts.append(xh_t)
                    # transpose x_hi/x_lo 128-blocks for the gating lhsT
                    xTh = xt.tile([P, M], BF16, tag="xTh", name=f"xTh{sb}")
                    xTl = xt.tile([P, M], BF16, tag="xTl", name=f"xTl{sb}")
                    for half, (srct, dstt) in enumerate(((xh_t, xTh), (xl_t, xTl))):
                        for q8 in range(2):
                            pst = psA.tile([P, 8 * P], BF16, tag="pst", bufs=3,
                                           name=f"pst{sb}_{half}_{q8}")
                            for r in range(8):
                                mo = q8 * 8 + r
                                nc.tensor.transpose(
                                    pst[:, ts(r, P)], srct[:, ts(mo, P)], ident_b[:])
                            if half == 0:
                                nc.vector.tensor_copy(dstt[:, ts(q8, 8 * P)], pst[:])
                            else:
                                nc.scalar.copy(dstt[:, ts(q8, 8 * P)], pst[:])
                    # gating logits: accumulate x_hi@Wg_hi + x_hi@Wg_lo +
                    # x_lo@Wg_hi + bg into ONE psum region (avoids any
                    # two-PSUM-operand consolidation op)
                    for mo in range(MO):
                        nc.tensor.matmul(psG[:, j, :], lhsT=xTh[:, ts(mo, P)],
                                         rhs=wg_sb[:, mo, :E],
                                         start=(mo == 0), stop=False)
                    for mo in range(MO):
                        nc.tensor.matmul(psG[:, j, :], lhsT=xTh[:, ts(mo, P)],
                                         rhs=wg_sb[:, mo, E:],
                                         start=False, stop=False)
                    for mo in range(MO):
                        nc.tensor.matmul(psG[:, j, :], lhsT=xTl[:, ts(mo, P)],
                                         rhs=wg_sb[:, mo, :E],
                                         start=False, stop=False)
                    nc.tensor.matmul(psG[:, j, :], lhsT=ones1[:], rhs=bg_sb[:, :E],
                                     start=False, stop=True)

                # ---- batched top-2 over the KB token blocks ----
                def t3(tag, w=E, dt=F32):
                    return sp.tile([P, KB, w], dt, tag=tag, name=f"{tag}{grp}")
                # probs = exp(logits); |logits| <~ 7 so no max-shift needed
                probs = t3("probs")
                nc.scalar.activation(probs[:], psG[:], AF.Exp)
                sume = t3("sume", 1)
                nc.vector.tensor_reduce(sume[:], probs[:], axis=AX.X, op=ALU.add)
                rcp = t3("rcp", 1)
                nc.vector.reciprocal(rcp[:], sume[:])
                p1 = t3("p1", 1)
                nc.vector.tensor_reduce(p1[:], probs[:], axis=AX.X, op=ALU.max)
                oh1 = t3("oh1")
                a, b = broadcast_tensor_aps(probs[:], p1[:])
                nc.vector.tensor_tensor(oh1[:], a, b, ALU.is_equal)
                noh = t3("noh")
                nc.vector.tensor_scalar(noh[:], oh1[:], -1.0, 1.0,
                                        op0=ALU.mult, op1=ALU.add)
                pm = t3("pm")
                nc.vector.tensor_tensor(pm[:], probs[:], noh[:], ALU.mult)
                p2 = t3("p2", 1)
                nc.vector.tensor_reduce(p2[:], pm[:], axis=AX.X, op=ALU.max)
                oh2 = t3("oh2")
                a, b = broadcast_tensor_aps(pm[:], p2[:])
                nc.vector.tensor_tensor(oh2[:], a, b, ALU.is_equal)
                # w1 = 2*sigmoid((p1-p2)*rcp) = 2/(1+e), e = exp((p2-p1)*rcp)
                d12 = t3("d12", 1)
                nc.vector.tensor_tensor(d12[:], p2[:], p1[:], ALU.subtract)
                dr = t3("dr", 1)
                nc.vector.tensor_tensor(dr[:], d12[:], rcp[:], ALU.mult)
                ex = t3("ex", 1)
                nc.scalar.activation(ex[:], dr[:], AF.Exp)
                one = t3("one", 1)
                nc.vector.tensor_scalar(one[:], ex[:], 1.0, None, op0=ALU.add)
                rr2 = t3("rr2", 1)
                nc.vector.reciprocal(rr2[:], one[:])
                er2 = t3("er2", 1)
                nc.vector.tensor_tensor(er2[:], ex[:], rr2[:], ALU.mult)
                t1 = t3("t1")
                a, b = broadcast_tensor_aps(oh1[:], rr2[:])
                nc.vector.tensor_tensor(t1[:], a, b, ALU.mult)
                t2 = t3("t2")
                a, b = broadcast_tensor_aps(oh2[:], er2[:])
                nc.vector.tensor_tensor(t2[:], a, b, ALU.mult)
                cw_b = t3("cwb", dt=BF16)
                nc.vector.tensor_tensor(cw_b[:], t1[:], t2[:], ALU.add)
                nc.vector.tensor_scalar(cw_b[:], cw_b[:], 2.0, None, op0=ALU.mult)
                dm_b = t3("dmb", dt=BF16)
                nc.vector.tensor_tensor(dm_b[:], oh1[:], oh2[:], ALU.add)

                # dispatch: D[e,m] += dm[s,e]^T @ x_hi[s,m] (bank-sized chunks)
                for j, sb in enumerate(sbs):
                    for c in range(M // 512):
                        nc.tensor.matmul(
                            d_ps[:, ts(c, 512)], lhsT=dm_b[:, j, :],
                            rhs=xh_ts[j][:, ts(c, 512)],
                            start=(sb == 0), stop=(sb == SB - 1),
                        )
                # cw^T into [E, S] layout for the combine matmul
                pcw = psA.tile([P, 8 * P], BF16, tag="pst", bufs=3,
                               name=f"pcw{grp}")
                for j in range(KB):
                    nc.tensor.transpose(pcw[:E, ts(j, P)], cw_b[:, j, :],
                                        ident_b[:])
                nc.scalar.copy(cwT_sb[:, grp * KB:(grp + 1) * KB, :],
                               pcw[:E, :KB * P])

            # dispatch AllToAll: row e -> core e; receive [G, M] for my expert
            dstage = keep_pool.tile([E, M], BF16, name="dstage")
            nc.vector.tensor_copy(dstage[:, :M // 2], d_ps[:, :M // 2])
            nc.scalar.copy(dstage[:, M // 2:], d_ps[:, M // 2:])
            nc.gpsimd.dma_start(d_in[:], dstage[:])
            nc.gpsimd.collective_compute(
                "AllToAll", ALU.bypass, replica_groups=rg,
                ins=[d_in.opt()], outs=[d_out.opt()],
            )
            nc.gpsimd.dma_start(dstage[:], d_out[:])
            for q4 in range(4):
                pd = psA.tile([P, 8 * P], BF16, tag="pst", bufs=3, name=f"pd{q4}")
                for r in range(4):
                    mo = q4 * 4 + r
                    nc.tensor.transpose(pd[:, r * E:(r + 1) * E],
                                        dstage[:, ts(mo, P)], ident_b[:E, :E])
                nc.vector.tensor_copy(dt_sb[:, q4 * 4:(q4 + 1) * 4, :],
                                      pd[:, :4 * E])

        # ---------- phase B: expert FFN + combine (expert-parallel) ----------
        with (
            tc.tile_pool(name="sp2", bufs=2) as sp2,
            tc.tile_pool(name="outp", bufs=2) as outp,
            tc.tile_pool(name="psB", bufs=2, space="PSUM") as psB,
            tc.tile_pool(name="psH", bufs=2, space="PSUM") as psH,
            tc.tile_pool(name="psC", bufs=2, space="PSUM") as psC,
        ):
            # FFN1: h[g, hslab] = D[g,:] @ wi_slab, silu, transpose into ht_sb
            dmy1 = psB.tile([E, E], F32, tag="psh", name="dmy1")
            nc.tensor.matmul(dmy1[:], lhsT=dt_sb[:, MO - 1, :], rhs=dt_sb[:, MO - 1, :],
                             start=True, stop=True)
            for hs in range(NSUP):
                wt = wi_pool.tile([P, MO * HSUP], BF16, tag="wi", name=f"wi{hs}")
                nc.scalar.dma_start(wt[:], wiT[ts(hs, P), :])
                ps_h = psB.tile([E, 512], F32, tag="psh", name=f"psh{hs}")
                for mo in range(MO):
                    nc.tensor.matmul(
                        ps_h[:], lhsT=dt_sb[:, mo, :], rhs=wt[:, ts(mo, 512)],
                        start=(mo == 0), stop=(mo == MO - 1),
                    )
                sgh = sp2.tile([E, 512], F32, tag="sgh", name=f"sgh{hs}")
                nc.scalar.activation(sgh[:], ps_h[:], AF.Sigmoid)
                h_b = sp2.tile([E, 512], BF16, tag="hsb", name=f"h{hs}")
                nc.vector.tensor_tensor(h_b[:], ps_h[:], sgh[:], ALU.mult)
                ho0 = hs * 4
                pht = psH.tile([P, 4 * E], BF16, tag="psht", name=f"pht{hs}")
                for q in range(4):
                    nc.tensor.transpose(pht[:, q * E:(q + 1) * E],
                                        h_b[:, ts(q, P)], ident_b[:E, :E])
                nc.vector.tensor_copy(ht_sb[:, ho0:ho0 + 4, :], pht[:, :4 * E])

            # FFN2 + AllToAll + combine + output, pipelined per m-chunk
            for mc in range(MC):
                ps_eo = psB.tile([E, MCH], F32, tag="psh", name=f"pseo{mc}")
                for wh in range(WOH):
                    wo_t = wo_pool.tile([P, OHB * MCH], BF16, tag="wo",
                                        name=f"wo{mc}_{wh}")
                    nc.sync.dma_start(wo_t[:], woT[ts(mc * WOH + wh, P), :])
                    for o in range(OHB):
                        ho = wh * OHB + o
                        nc.tensor.matmul(
                            ps_eo[:], lhsT=ht_sb[:, ho, :], rhs=wo_t[:, ts(o, MCH)],
                            start=(ho == 0), stop=(ho == HB - 1),
                        )
                eo_sb = sp2.tile([E, MCH], BF16, tag="eosb", name=f"eo{mc}")
                nc.vector.tensor_copy(eo_sb[:], ps_eo[:])
                nc.gpsimd.dma_start(eo_in[mc][:], eo_sb[:])
                nc.gpsimd.collective_compute(
                    "AllToAll", ALU.bypass, replica_groups=rg,
                    ins=[eo_in[mc].opt()], outs=[eo_out[mc].opt()],
                )
                eoall = sp2.tile([E, MCH], BF16, tag="eoall", name=f"eoall{mc}")
                nc.gpsimd.dma_start(eoall[:], eo_out[mc][:])
                for sb in range(SB):
                    ps_o = psC.tile([P, MCH], F32, tag="pso", name=f"pso{mc}_{sb}")
                    nc.tensor.matmul(
                        ps_o[:], lhsT=cwT_sb[:, sb, :], rhs=eoall[:],
                        start=True, stop=True,
                    )
                    if sb % 2 == 0:
                        ob = outp.tile([P, 2 * MCH], BF16, tag="ob",
                                       name=f"ob{mc}_{sb // 2}")
                        nc.vector.tensor_copy(ob[:, ts(0, MCH)], ps_o[:])
                    else:
                        nc.scalar.copy(ob[:, ts(1, MCH)], ps_o[:])
                        nc.sync.dma_start(
                            out[:, (sb - 1):(sb + 1), ts(mc, MCH)], ob[:])

    nc.finalize()
    return nc


def prepare_in_maps(x, Wg, bg, wi, wo):
    G, S, M = x.shape
    E, H, _ = wi.shape
    SB, MO, HB = S // P, M // P, H // P
    HSUP, MCH, WOH = 512, 512, 4
    NSUP, MC = H // HSUP, M // MCH
    OHB = HB // WOH

    Wg32 = np.asarray(Wg, dtype=np.float32)
    Wg_hi = Wg32.astype(ml_dtypes.bfloat16)
    Wg_lo = (Wg32 - Wg_hi.astype(np.float32)).astype(ml_dtypes.bfloat16)
    # [P, MO, 2E]: [:, mo, :E] = Wg_hi block, [:, mo, E:] = Wg_lo block
    wg_arr = np.concatenate(
        [Wg_hi.reshape(MO, P, E).transpose(1, 0, 2),
         Wg_lo.reshape(MO, P, E).transpose(1, 0, 2)], axis=2)
    wg_arr = np.ascontiguousarray(wg_arr)
    bg_arr = np.zeros((1, 2 * E), dtype=np.float32)
    bg_arr[0, :E] = np.asarray(bg, dtype=np.float32)

    in_maps = []
    for c in range(N_CORES):
        xc = np.asarray(x[c], dtype=np.float32)
        x_hi = xc.astype(ml_dtypes.bfloat16)
        x_lo = (xc - x_hi.astype(np.float32)).astype(ml_dtypes.bfloat16)
        # wiT slabs [NSUP*P, MO*HSUP]:
        #   row hs*P+p, col mo*HSUP+h' = wi[c][hs*HSUP+h', mo*P+p]
        wiT_c = np.ascontiguousarray(wi[c].T).astype(ml_dtypes.bfloat16)  # [M, H]
        wiT_c = np.ascontiguousarray(
            wiT_c.reshape(MO, P, NSUP, HSUP).transpose(2, 1, 0, 3)
            .reshape(NSUP * P, MO * HSUP))
        # woT slabs [(MC*WOH)*P, OHB*MCH]: slab k=mc*WOH+wh holds
        #   row k*P+p, col o*MCH+m' = wo[c][mc*MCH+m', (wh*OHB+o)*P+p]
        woT_c = np.ascontiguousarray(wo[c].T).astype(ml_dtypes.bfloat16)  # [H, M]
        woT_c = np.ascontiguousarray(
            woT_c.reshape(WOH, OHB, P, MC, MCH).transpose(3, 0, 2, 1, 4)
            .reshape(MC * WOH * P, OHB * MCH))
        in_maps.append({
            "xh": np.ascontiguousarray(x_hi),
            "xl": np.ascontiguousarray(x_lo),
            "wgc": wg_arr,
            "bgp": bg_arr,
            "wiT": wiT_c,
            "woT": woT_c,
        })
    return in_maps


def kernel(x, Wg, bg, wi, wo):
    global LAST_RESULT
    from concourse.bass_utils import run_bass_kernel_spmd

    x = np.asarray(x); Wg = np.asarray(Wg); bg = np.asarray(bg)
    wi = np.asarray(wi); wo = np.asarray(wo)
    nc = build_bass()
    in_maps = prepare_in_maps(x, Wg, bg, wi, wo)
    try:
        res = run_bass_kernel_spmd(
            nc, in_maps, core_ids=list(range(N_CORES)),
            trace=bool(int(os.environ.get("MOE_TRACE", "0"))),
        )
    except ModuleNotFoundError:
        # NTFF profiling hook unavailable in this environment — run untraced.
        os.environ["BASS_NEVER_TRACE"] = "1"
        res = run_bass_kernel_spmd(nc, in_maps, core_ids=list(range(N_CORES)))
    LAST_RESULT = res
    S, M = x.shape[1], x.shape[2]
    outs = []
    for r in res.results:
        o = np.asarray(r["out"])              # [P, SB, M] bf16
        outs.append(o.transpose(1, 0, 2).reshape(S, M).astype(np.float32))
    return np.stack(outs)
